# revision 32
# baseline (speedup 1.0000x reference)
"""CWRNN language-model kernel for 8 Trainium2 NeuronCores.

Strategy (vocab-sharded output projection):
  - logits = scan(h) @ Wo with V=32000 dominates: the fp32 output
    [16, 255, 32000] is 522 MB of HBM writes.  Each core gets Wo[:, c*4000 :
    (c+1)*4000], runs the (tiny, serial) recurrence redundantly, and writes its
    own [16, 255, 4000] slice.  No collectives.
  - The serial recurrence is the latency limiter, so its per-step chain is cut
    to two engine hops: PE accumulates U'_t into PSUM via an identity matmul
    (no dependence on h, runs early) then adds Wh·h_{t-1} on top; ACT applies
    tanh and writes the new state DIRECTLY into the t-major history tile
    column that the next step's matmul and the output projection read.  No
    per-step state copies sit on the chain.
  - Clock blocks that hold their value (periods 2/4/8) are materialized into
    the skipped history columns by small DVE copies / GpSimd broadcasts that
    depend only on an older column -> fully off the critical chain.
  - All PSUM drains run on DVE; ACT does nothing but the 319 tanhs.
  - Projection chunks (250 vocab cols) are emitted 2 per recurrence step so
    the PE stream stays dense (HAM warm) but any single matmul blocks the
    chain's next step by at most ~110 ns.
"""

import sys

sys.path.insert(0, "/opt/trn_rl_repo")

import numpy as np

import concourse.bass as bass
import concourse.mybir as mybir
import concourse.tile as tile
from concourse.tile import add_dep_helper
from concourse import bacc
from concourse.bass_utils import run_bass_kernel_spmd
from concourse.masks import make_identity

F32 = mybir.dt.float32
F16 = mybir.dt.float16
I32 = mybir.dt.int32
I16 = mybir.dt.int16
TANH = mybir.ActivationFunctionType.Tanh

B = 16
T = 255           # x[:, :-1]
E = 256
NH = 256
V = 32000
NCORES = 8
VS = V // NCORES  # 4000 vocab columns per core
TOK = B * T       # 4080 tokens
NT = 32           # token tiles of 128 (31 full + 1 of 112)
VC = 500          # vocab chunk per projection matmul
NVC = VS // VC    # 8 chunks per token tile


def build_program():
    nc = bacc.Bacc(target_bir_lowering=False)

    d_ids = nc.dram_tensor("tok_ids", [128, NT], I32, kind="ExternalInput")
    d_emb = nc.dram_tensor("embedding", [V + 1, E], F32, kind="ExternalInput")
    d_wi = nc.dram_tensor("Wi", [E, NH], F32, kind="ExternalInput")
    d_wh = nc.dram_tensor("Wh", [NH, NH], F32, kind="ExternalInput")
    d_bi = nc.dram_tensor("bi", [NH], F32, kind="ExternalInput")
    d_bh = nc.dram_tensor("bh", [NH], F32, kind="ExternalInput")
    d_wo = nc.dram_tensor("Wo_s", [NH, VS], F32, kind="ExternalInput")
    d_out = nc.dram_tensor("out", [NT * 128, VS], F16, kind="ExternalOutput")

    with tile.TileContext(nc) as tc:
        with tc.tile_pool(name="const", bufs=1) as cpool, \
             tc.tile_pool(name="uprime", bufs=1) as upool, \
             tc.tile_pool(name="hist", bufs=1) as hpool, \
             tc.tile_pool(name="obuf", bufs=8) as opool, \
             tc.tile_pool(name="work", bufs=3) as wpool, \
             tc.tile_pool(name="psum", bufs=2, space="PSUM") as psum:

            # ---------------- constants and weights ----------------
            ids_sb = cpool.tile([128, NT], I32)
            nc.sync.dma_start(out=ids_sb[:], in_=d_ids[:])

            wh16 = []
            wi16 = []

            def emit_weight_loads():
                for k in range(2):
                    # SWDGE casts fp32 -> fp16 during the transfer
                    w = cpool.tile([128, NH], F16, name=f"wh16_{k}")
                    nc.gpsimd.dma_start(out=w[:], in_=d_wh[k * 128:(k + 1) * 128, :])
                    wh16.append(w)
                    wi = cpool.tile([128, NH], F16, name=f"wi16_{k}")
                    nc.gpsimd.dma_start(out=wi[:], in_=d_wi[k * 128:(k + 1) * 128, :])
                    wi16.append(wi)

            bias = []
            for m in range(2):
                bi_t = cpool.tile([128, 1], F32, name=f"bi{m}")
                bh_t = cpool.tile([128, 1], F32, name=f"bh{m}")
                nc.sync.dma_start(out=bi_t[:], in_=d_bi[m * 128:(m + 1) * 128, None])
                nc.sync.dma_start(out=bh_t[:], in_=d_bh[m * 128:(m + 1) * 128, None])
                bs = cpool.tile([128, 1], F32, name=f"bias{m}")
                nc.vector.tensor_add(bs[:], bi_t[:], bh_t[:])
                bias.append(bs)

            ident16 = cpool.tile([128, 128], F16)
            make_identity(nc, ident16[:])

            # history h^T tiles, fp16, t-major: col = t_local*16 + b.
            # htA rows = units 0:128 (clock blocks 0,1), htB rows = units
            # 128:256 (blocks 2,3).  tanh writes update columns; held values
            # are filled in by off-chain copies.
            htA = [hpool.tile([128, 128], F16, name=f"htA_{g}") for g in range(NT)]
            htB = [hpool.tile([128, 128], F16, name=f"htB_{g}") for g in range(NT)]
            # tile 31 col 7 is never written; its ob rows are discarded, but
            # keep the values finite
            nc.vector.memset(htA[NT - 1][:, 112:128], 0.0)
            nc.vector.memset(htB[NT - 1][:, 112:128], 0.0)

            # U' = emb @ Wi + (bi+bh), fp16, same t-major col layout
            u16 = [[upool.tile([128, 128], F16, tag=f"u{m}", bufs=NT,
                               name=f"u{m}_{g}") for g in range(NT)]
                   for m in range(2)]

            # ---------------- phase 1+2: gather -> embT -> U' ----------------
            embt_of = {}

            # The Tile scheduler orders each engine's stream by its own
            # priorities, not emission order.  Left alone it hoists whole
            # phases (all gathers+transposes in one burst) in front of the
            # serial recurrence and starves it.  pace() adds an
            # ordering-only edge tying every off-chain instruction behind
            # the most recent recurrence step.
            last_tanh = {"ins": None}

            def pace(bi):
                if last_tanh["ins"] is not None:
                    add_dep_helper(bi.ins, last_tanh["ins"], sync=False,
                                   reason="pace off-chain work after recurrence")
                return bi

            g16_of = {}
            # PE-stream phase-1/2 instructions whose inputs may lag (gather
            # cast): anchored behind the NEXT step's recurrence matmul so an
            # unready transpose can never idle the in-order PE ahead of the
            # serial chain
            defer_after_mmh = []

            def emit_gather(g: int):
                gth = wpool.tile([128, E], F32, tag="gather", bufs=4,
                                 name=f"gth_{g}")
                pace(nc.gpsimd.indirect_dma_start(
                    out=gth[:], out_offset=None, in_=d_emb[:],
                    in_offset=bass.IndirectOffsetOnAxis(ap=ids_sb[:, g:g + 1], axis=0),
                ))
                g16 = wpool.tile([128, E], F16, tag="g16", bufs=4, name=f"g16_{g}")
                pace(nc.gpsimd.tensor_copy(g16[:], gth[:]))
                g16_of[g] = g16

            def emit_transposes(g: int):
                g16 = g16_of.pop(g)
                embt = []
                for k in range(2):
                    tp = psum.tile([128, 128], F16, tag="tp", bufs=1, space="PSUM",
                                   name=f"tp_{g}_{k}")
                    tr = pace(nc.tensor.transpose(
                        out=tp[:], in_=g16[:, k * 128:(k + 1) * 128],
                        identity=ident16[:]))
                    defer_after_mmh.append(tr.ins)
                    et = wpool.tile([128, 128], F16, tag=f"embt{k}", bufs=4,
                                    name=f"et_{g}_{k}")
                    pace(nc.vector.tensor_copy(et[:], tp[:]))
                    embt.append(et)
                embt_of[g] = embt

            def emit_uprime(g: int):
                embt = embt_of.pop(g)
                for m in range(2):
                    up = psum.tile([128, 128], F32, tag="up", bufs=1, space="PSUM",
                                   name=f"up_{g}_{m}")
                    for k in range(2):
                        um = pace(nc.tensor.matmul(
                            out=up[:], lhsT=wi16[k][:, m * 128:(m + 1) * 128],
                            rhs=embt[k][:], start=(k == 0), stop=(k == 1)))
                        defer_after_mmh.append(um.ins)
                    # fold bias during the fp32 -> fp16 PSUM drain
                    pace(nc.vector.tensor_scalar_add(u16[m][g][:], up[:], bias[m][:]))

            emit_gather(0)
            emit_weight_loads()
            for _g in range(1, 4):
                emit_gather(_g)
            for _g in range(4):
                emit_transposes(_g)
                emit_uprime(_g)

            # Wo cast-loads are emitted at t==1 (paced) so they cannot be
            # scheduled ahead of the first gathers on the SWDGE queues; the
            # projection first needs them ~8 steps in.
            wo16 = []

            def emit_wo_loads():
                for k in range(2):
                    wo = cpool.tile([128, VS], F16, name=f"wo16_{k}")
                    pace(nc.gpsimd.dma_start(
                        out=wo[:], in_=d_wo[k * 128:(k + 1) * 128, :]))
                    wo16.append(wo)

            emit_wo_loads()

            # the upfront phase-1/2 feeds the first steps directly; only
            # in-loop prefetch instructions defer behind the chain
            defer_after_mmh.clear()

            # ---------------- phase 3+4: recurrence + projection ----------------
            ob_tiles = {}
            done_chunks = {}
            from collections import deque
            pending = deque()
            chunk_mms = deque()  # proj mms awaiting an ordering anchor

            def emit_chunk(g: int, vc: int):
                if g not in ob_tiles:
                    ob_tiles[g] = opool.tile([128, VS], F16, tag="ob", name=f"ob_{g}")
                    done_chunks[g] = 0
                ob = ob_tiles[g]
                pp = psum.tile([128, VC], F32, tag="pp", bufs=3, space="PSUM")
                pace(nc.tensor.matmul(out=pp[:], lhsT=htA[g][:],
                                      rhs=wo16[0][:, vc * VC:(vc + 1) * VC],
                                      start=True, stop=False))
                last_mm = pace(nc.tensor.matmul(
                    out=pp[:], lhsT=htB[g][:],
                    rhs=wo16[1][:, vc * VC:(vc + 1) * VC],
                    start=False, stop=True))
                chunk_mms.append(last_mm.ins)
                pace(nc.vector.tensor_copy(ob[:, vc * VC:(vc + 1) * VC], pp[:]))
                done_chunks[g] += 1
                if done_chunks[g] == NVC:
                    # contiguous 1 MB destination row-block; round-robin the
                    # three DMA issue paths so transfers overlap across queues
                    dma_eng = (nc.sync, nc.scalar)[g % 2]
                    rows = 128 if g < NT - 1 else 112
                    pace(dma_eng.dma_start(
                        out=d_out[g * 128:g * 128 + rows, :],
                        in_=ob[:rows, :]))
                    del ob_tiles[g]

            for t in range(T):
                g, c = divmod(t, 8)
                uc = slice(c * B, (c + 1) * B)

                # ---- recurrence step t ----
                if t == 0:
                    # h_prev = 0: h = tanh(U'_0) for all 256 units
                    th = nc.scalar.activation(htA[0][:, uc], u16[0][0][:, uc], TANH)
                    nc.scalar.activation(htB[0][:, uc], u16[1][0][:, uc], TANH)
                    last_tanh["ins"] = th.ins
                else:
                    pg = g if c > 0 else g - 1
                    pc = (c - 1) % 8
                    up_ = slice(pc * B, (pc + 1) * B)
                    prevA = htA[pg]
                    prevB = htB[pg]

                    # out units 0:rowsA of tile A (block 0 always; block 1 on
                    # even t).  The clock mask baked into Wh zeroes the
                    # block1->block0 rows, so one 128-contraction matmul over
                    # the full current-state column is correct for both.
                    rowsA = 128 if t % 2 == 0 else 64
                    pa = psum.tile([128, B], F32, tag="pa", bufs=2, space="PSUM",
                                   name=f"pa_{t}")
                    ia_mm = nc.tensor.matmul(out=pa[:rowsA, :],
                                             lhsT=ident16[:rowsA, :rowsA],
                                             rhs=u16[0][g][:rowsA, uc],
                                             start=True, stop=False)
                    for d in defer_after_mmh:
                        add_dep_helper(d, ia_mm.ins, sync=False,
                                       reason="phase12 behind the chain matmul")
                    defer_after_mmh.clear()
                    if len(chunk_mms) > 2 and t > 16:
                        # ordering-only edge: force the scheduler to keep the
                        # projection stream interleaved with the recurrence
                        # instead of deferring it wholesale
                        add_dep_helper(ia_mm.ins, chunk_mms.popleft(),
                                       sync=False,
                                       reason="pace projection inside recurrence")
                    nc.tensor.matmul(out=pa[:rowsA, :],
                                     lhsT=wh16[0][:, :rowsA],
                                     rhs=prevA[:, up_],
                                     start=False, stop=True)
                    th = nc.scalar.activation(htA[g][:rowsA, uc], pa[:rowsA, :],
                                              TANH)
                    last_tanh["ins"] = th.ins

                    if t % 4 == 0:
                        # out units 128:128+rowsB of tile B (block 2; block 3
                        # on t%8==0); contraction over all 256 units, mask
                        # handles sub-block feeds.
                        rowsB = 128 if t % 8 == 0 else 64
                        pb = psum.tile([128, B], F32, tag="pb", bufs=1,
                                       space="PSUM", name=f"pb_{t}")
                        nc.tensor.matmul(out=pb[:rowsB, :],
                                         lhsT=ident16[:rowsB, :rowsB],
                                         rhs=u16[1][g][:rowsB, uc],
                                         start=True, stop=False)
                        nc.tensor.matmul(out=pb[:rowsB, :],
                                         lhsT=wh16[0][:, 128:128 + rowsB],
                                         rhs=prevA[:, up_],
                                         start=False, stop=False)
                        nc.tensor.matmul(out=pb[:rowsB, :],
                                         lhsT=wh16[1][:, 128:128 + rowsB],
                                         rhs=prevB[:, up_],
                                         start=False, stop=True)
                        nc.scalar.activation(htB[g][:rowsB, uc], pb[:rowsB, :],
                                             TANH)

                    # ---- held-state fill-in (off the serial chain: each op
                    # reads a column written >= 1 step ago) ----
                    if t % 2 == 1:
                        # block 1 holds: copy its rows into this step's column
                        nc.scalar.copy(htA[g][64:128, uc],
                                       prevA[64:128, up_])
                    if t % 4 == 1:
                        # block 2 held for the next 3 columns
                        span = min(3, 8 - c)
                        nc.gpsimd.tensor_copy(
                            htB[g][:64, c * B:(c + span) * B].rearrange(
                                "p (t b) -> p t b", b=B),
                            prevB[:64, up_][:, None, :].to_broadcast(
                                [64, span, B]))
                    if c == 1:
                        # block 3 held for columns 1..7
                        nc.gpsimd.tensor_copy(
                            htB[g][64:, B:].rearrange("p (t b) -> p t b", b=B),
                            prevB[64:, up_][:, None, :].to_broadcast(
                                [64, 7, B]))

                # ---- prefetch pipeline: gather 2 groups ahead-of-time,
                # transpose/U' per group with a 3-group lead ----
                if c == 0 and g + 4 < NT:
                    emit_gather(g + 4)
                if c == 2 and g + 4 < NT:
                    emit_transposes(g + 4)
                elif c == 3 and g + 4 < NT:
                    emit_uprime(g + 4)

                # ---- projection of the previous group, 2 chunks per step ----
                if c == 7:
                    pending.extend((g, vc) for vc in range(NVC))
                if pending:
                    emit_chunk(*pending.popleft())

            pending.extend((NT - 1, vc) for vc in range(NVC))
            while pending:
                emit_chunk(*pending.popleft())

    nc.finalize()
    return nc


_NC_CACHE = None
TRACE = False        # set by test harness to capture an NTFF profile
TRACE_KW = {}
LAST_RESULT = None   # BassKernelResults of the most recent run


def kernel(x, x_sl, embedding, Wi, Wh, bi, bh, Wo):
    global _NC_CACHE, LAST_RESULT
    if _NC_CACHE is None:
        _NC_CACHE = build_program()
    nc = _NC_CACHE

    x = np.asarray(x)
    ids = np.ascontiguousarray(x[:, :T].T).reshape(-1)  # n = t*B + b -> x[b, t]
    ids_pad = np.zeros(128 * NT, np.int32)
    ids_pad[:TOK] = ids
    ids_dev = np.ascontiguousarray(ids_pad.reshape(NT, 128).T)

    embedding = np.ascontiguousarray(np.asarray(embedding, np.float32))
    Wi = np.ascontiguousarray(np.asarray(Wi, np.float32))
    Wh = np.ascontiguousarray(np.asarray(Wh, np.float32))
    bi = np.ascontiguousarray(np.asarray(bi, np.float32))
    bh = np.ascontiguousarray(np.asarray(bh, np.float32))
    Wo = np.asarray(Wo, np.float32)

    in_maps = []
    for c in range(NCORES):
        in_maps.append({
            "tok_ids": ids_dev,
            "embedding": embedding,
            "Wi": Wi,
            "Wh": Wh,
            "bi": bi,
            "bh": bh,
            "Wo_s": np.ascontiguousarray(Wo[:, c * VS:(c + 1) * VS]),
        })

    res = run_bass_kernel_spmd(nc, in_maps, core_ids=list(range(NCORES)),
                               trace=TRACE, **TRACE_KW)
    LAST_RESULT = res
    # device writes fp16 logits t-major (row = g*128 + t_local*16 + b) for
    # contiguous DMA; reassemble + upcast on host
    parts = []
    for r in res.results:
        o = r["out"].reshape(NT, 8, B, VS).transpose(2, 0, 1, 3).reshape(B, NT * 8, VS)
        parts.append(o[:, :T, :])
    return np.concatenate(parts, axis=2).astype(np.float32)
